# revision 1
# baseline (speedup 1.0000x reference)
"""CommNet message-passing kernel for Trainium2 (8 NeuronCores).

Problem (reference semantics):
    A, B, S, H = 8, 64, 1024, 128
    msg   = transpose(rnn_h, (2,1,0,3)) * alive            # (A,B,S,H)
    denom = max(sum_a alive, 1)                            # (1,B,S,1)
    msg   = msg / denom
    msg   = einsum('absh,oh->abso', msg, W) + b            # per-token HxH linear
    out   = obs + msg.reshape(A*B, S, H)

Sharding: data-parallel over the env-batch axis B (8 batches per core).
All ops are batch-local; W/b are replicated.

Per-core kernel layout strategy:
  - tokens are ordered (a, b, s) to match obs/out memory order, so the obs
    load and out store are contiguous; the rnn_h load is the strided stream
    (512B chunks) and implements the (S,B,A,H)->(A,B,S,H) permute.
  - per (a,b) pair: 1024 tokens = 8 sub-tiles of 128 tokens.
    Each 512-token group: pre-scale by alive/denom (DVE per-partition
    scalars), PE-transpose to (H, tokens), one W-stationary float32r matmul
    (N=512 -> full rate), bias added on ScalarE during the PSUM->SBUF copy,
    PE-transpose back to (tokens, H), residual add with obs on DVE.
  - alive -> scale = alive/max(sum_a alive,1) is computed on device with two
    small selector matmuls (partition-axis reduce + broadcast) and 8 PE
    transposes into the (token-partition, tile-column) layout.
"""

import os
import sys

import numpy as np

for _p in ("/opt/trn_rl_repo", "/root/.axon_site/_ro/trn_rl_repo"):
    if os.path.isdir(_p) and _p not in sys.path:
        sys.path.append(_p)

A, B, S, H = 8, 64, 1024, 128
NCORES = 8
BLOC = B // NCORES  # 8 env batches per core

F32 = None  # set lazily after imports


def _build_program(s_len=S, transpose_dt="float32", reps=1):
    """Build the per-core Bass program (identical on all cores).

    reps>1 repeats the whole main loop (same I/O) — used only for timing,
    since single-call wall time is dominated by ~70ms axon RTT."""
    import concourse.bass as bass  # noqa: F401
    import concourse.bacc as bacc
    import concourse.tile as tile
    from concourse import mybir

    f32 = mybir.dt.float32
    f32r = mybir.dt.float32r
    i32 = mybir.dt.int32

    assert s_len % 512 == 0
    nj = s_len // 128       # sub-tiles per (a,b)
    ngroups = s_len // 512  # 512-token groups per (a,b)

    nc = bacc.Bacc("TRN2", target_bir_lowering=False, debug=False,
                   num_devices=NCORES)

    rnn = nc.dram_tensor("rnn", [s_len, BLOC, A, H], f32,
                         kind="ExternalInput").ap()
    obs = nc.dram_tensor("obs", [A, BLOC, s_len, H], f32,
                         kind="ExternalInput").ap()
    alive = nc.dram_tensor("alive", [A, BLOC, s_len], i32,
                           kind="ExternalInput").ap()
    wt = nc.dram_tensor("wt", [H, H], f32, kind="ExternalInput").ap()
    bias = nc.dram_tensor("bias", [H, 1], f32, kind="ExternalInput").ap()
    ident = nc.dram_tensor("ident", [128, 128], f32, kind="ExternalInput").ap()
    sel = nc.dram_tensor("sel", [64, 8], f32, kind="ExternalInput").ap()
    sel2 = nc.dram_tensor("sel2", [8, 64], f32, kind="ExternalInput").ap()
    out = nc.dram_tensor("out", [A, BLOC, s_len, H], f32,
                         kind="ExternalOutput").ap()

    # tokens (p within sub-tile j) views
    rnn_r = rnn.rearrange("(j p) b a h -> b a p j h", p=128)
    obs_r = obs.rearrange("a b (j p) h -> a b p j h", p=128)
    out_r = out.rearrange("a b (j p) h -> a b p j h", p=128)
    alive_r = alive.rearrange("a b s -> (a b) s")

    # dtype used for the transpose path (scaled msg, pa, ob, pc tiles).
    # float32r streams 1.5 cycles/row through the PE vs 2.0 for float32;
    # values are identical bits (transpose is routing; the producers round).
    tdt = {"float32": f32, "float32r": f32r,
           "bfloat16": mybir.dt.bfloat16}[transpose_dt]
    mm_dt = mybir.dt.bfloat16 if transpose_dt == "bfloat16" else f32r

    with tile.TileContext(nc) as tc:
        with tc.tile_pool(name="consts", bufs=1) as consts, \
             tc.tile_pool(name="pre", bufs=1) as pre, \
             tc.tile_pool(name="prepsum", bufs=1, space="PSUM") as prepsum, \
             tc.tile_pool(name="rnnp", bufs=4) as rnn_pool, \
             tc.tile_pool(name="obsp", bufs=4) as obs_pool, \
             tc.tile_pool(name="outp", bufs=4) as out_pool, \
             tc.tile_pool(name="scaledp", bufs=4) as scaled_pool, \
             tc.tile_pool(name="mtp", bufs=4) as mt_pool, \
             tc.tile_pool(name="obp", bufs=4) as ob_pool, \
             tc.tile_pool(name="pap", bufs=2, space="PSUM") as pa_pool, \
             tc.tile_pool(name="pbp", bufs=2, space="PSUM") as pb_pool, \
             tc.tile_pool(name="pcp", bufs=3, space="PSUM") as pc_pool:

            # ---- constants ----
            wt_sb = consts.tile([128, 128], f32, tag="wt")
            nc.sync.dma_start(out=wt_sb, in_=wt)
            # fp32r matmul operands must be *produced* as float32r (walrus
            # verifier); round W once on DVE.
            wt_r = consts.tile([128, 128], mm_dt, tag="wtr")
            nc.vector.tensor_copy(out=wt_r, in_=wt_sb)
            id_sb = consts.tile([128, 128], f32, tag="id")
            nc.sync.dma_start(out=id_sb, in_=ident)
            b_sb = consts.tile([128, 1], f32, tag="b")
            nc.sync.dma_start(out=b_sb, in_=bias)
            sel_sb = consts.tile([64, 8], f32, tag="sel")
            nc.sync.dma_start(out=sel_sb, in_=sel)
            sel2_sb = consts.tile([8, 64], f32, tag="sel2")
            nc.sync.dma_start(out=sel2_sb, in_=sel2)
            if tdt == f32:
                id_t = id_sb
            else:
                id_t = consts.tile([128, 128], tdt, tag="idt")
                nc.vector.tensor_copy(out=id_t, in_=id_sb)

            # ---- scale = alive / max(sum_a alive, 1) ----
            alive_i = pre.tile([64, s_len], i32, tag="alive_i")
            nc.sync.dma_start(out=alive_i, in_=alive_r)
            alive_f = pre.tile([64, s_len], f32, tag="alive_f")
            nc.vector.tensor_copy(out=alive_f, in_=alive_i)

            denom = pre.tile([8, s_len], f32, tag="denom")
            for hh in range(s_len // 512):
                dps = prepsum.tile([8, 512], f32, tag="pp")
                nc.tensor.matmul(out=dps, lhsT=sel_sb,
                                 rhs=alive_f[:, 512 * hh:512 * (hh + 1)],
                                 start=True, stop=True)
                nc.vector.tensor_scalar_max(
                    out=denom[:, 512 * hh:512 * (hh + 1)], in0=dps,
                    scalar1=1.0)
            inv = pre.tile([8, s_len], f32, tag="inv")
            nc.vector.reciprocal(out=inv, in_=denom)

            scale_nat = pre.tile([64, s_len], f32, tag="scale_nat")
            for hh in range(s_len // 512):
                ips = prepsum.tile([64, 512], f32, tag="pp")
                nc.tensor.matmul(out=ips, lhsT=sel2_sb,
                                 rhs=inv[:, 512 * hh:512 * (hh + 1)],
                                 start=True, stop=True)
                nc.vector.tensor_mul(
                    out=scale_nat[:, 512 * hh:512 * (hh + 1)],
                    in0=alive_f[:, 512 * hh:512 * (hh + 1)], in1=ips)

            # scale_sb[p, 64*j + (a*8+b)] = scale for token (a, b, 128*j+p)
            scps = prepsum.tile([128, 64 * nj], f32, tag="pp")
            for c in range(nj):
                nc.tensor.matmul(out=scps[:, 64 * c:64 * (c + 1)],
                                 lhsT=scale_nat[:, 128 * c:128 * (c + 1)],
                                 rhs=id_sb[:64, :64], is_transpose=True,
                                 start=(c == 0), stop=(c == nj - 1))
            scale_sb = pre.tile([128, 64 * nj], f32, tag="scale_sb")
            nc.vector.tensor_copy(out=scale_sb, in_=scps)

            # ---- main loop over (a, b) pairs ----
            ident_f = mybir.ActivationFunctionType.Identity
            for _rep in range(reps):
              for a in range(A):
                for b in range(BLOC):
                    ab = a * 8 + b
                    rnn_t = rnn_pool.tile([128, nj, 128], f32, tag="rnn_t")
                    nc.sync.dma_start(out=rnn_t, in_=rnn_r[b, a])
                    obs_t = obs_pool.tile([128, nj, 128], f32, tag="obs_t")
                    nc.sync.dma_start(out=obs_t, in_=obs_r[a, b])
                    out_t = out_pool.tile([128, nj, 128], f32, tag="out_t")

                    obs_fl = obs_t.rearrange("p j h -> p (j h)")
                    out_fl = out_t.rearrange("p j h -> p (j h)")

                    for g in range(ngroups):
                        scaled = scaled_pool.tile([128, 4, 128], tdt,
                                                  tag="scaled")
                        for jj in range(4):
                            j = 4 * g + jj
                            col = 64 * j + ab
                            nc.vector.tensor_scalar_mul(
                                out=scaled[:, jj, :], in0=rnn_t[:, j, :],
                                scalar1=scale_sb[:, col:col + 1])
                        pa = pa_pool.tile([128, 512], tdt, tag="pa")
                        for jj in range(4):
                            nc.tensor.matmul(
                                out=pa[:, 128 * jj:128 * (jj + 1)],
                                lhsT=scaled[:, jj, :],
                                rhs=id_t,
                                is_transpose=True,
                                start=(jj == 0), stop=(jj == 3))
                        mt = mt_pool.tile([128, 512], mm_dt, tag="mt")
                        nc.scalar.copy(out=mt, in_=pa)
                        pb = pb_pool.tile([128, 512], f32, tag="pb")
                        nc.tensor.matmul(out=pb, lhsT=wt_r, rhs=mt,
                                         start=True, stop=True)
                        ob = ob_pool.tile([128, 512], tdt, tag="ob")
                        nc.scalar.activation(out=ob, in_=pb, func=ident_f,
                                             bias=b_sb, scale=1.0)
                        pc = pc_pool.tile([128, 512], tdt, tag="pc")
                        for jj in range(4):
                            nc.tensor.matmul(
                                out=pc[:, 128 * jj:128 * (jj + 1)],
                                lhsT=ob[:, 128 * jj:128 * (jj + 1)],
                                rhs=id_t,
                                is_transpose=True,
                                start=(jj == 0), stop=(jj == 3))
                        nc.vector.tensor_add(
                            out=out_fl[:, 512 * g:512 * (g + 1)], in0=pc,
                            in1=obs_fl[:, 512 * g:512 * (g + 1)])
                    nc.sync.dma_start(out=out_r[a, b], in_=out_t)
    nc.compile()
    return nc


def make_in_maps(obs, rnn_h, alive, W, b, s_len=S):
    """Shard full inputs into per-core input maps (host-side slicing only)."""
    obs4 = obs.reshape(A, B, S, H)
    wt = np.ascontiguousarray(W.T.astype(np.float32))
    b2 = np.ascontiguousarray(b.astype(np.float32).reshape(H, 1))
    ident = np.eye(128, dtype=np.float32)
    sel = np.zeros((64, 8), np.float32)
    sel[np.arange(64), np.arange(64) % 8] = 1.0
    sel2 = np.ascontiguousarray(sel.T)
    in_maps = []
    for c in range(NCORES):
        bs = slice(BLOC * c, BLOC * (c + 1))
        in_maps.append({
            "rnn": np.ascontiguousarray(rnn_h[:s_len, bs]),
            "obs": np.ascontiguousarray(obs4[:, bs, :s_len]),
            "alive": np.ascontiguousarray(alive[:, bs, :s_len, 0]),
            "wt": wt, "bias": b2, "ident": ident, "sel": sel, "sel2": sel2,
        })
    return in_maps


_NC_CACHE = {}


def get_nc(s_len=S, transpose_dt="float32", reps=1):
    key = (s_len, transpose_dt, reps)
    if key not in _NC_CACHE:
        _NC_CACHE[key] = _build_program(s_len, transpose_dt, reps)
    return _NC_CACHE[key]


DEFAULT_TRANSPOSE_DT = "float32r"


def kernel(obs, rnn_h, alive, W, b):
    from concourse.bass_utils import run_bass_kernel_spmd

    nc = get_nc(S, DEFAULT_TRANSPOSE_DT)
    in_maps = make_in_maps(obs, rnn_h, alive, W, b)
    res = run_bass_kernel_spmd(nc, in_maps, list(range(NCORES))).results
    out = np.empty((A, B, S, H), np.float32)
    for c in range(NCORES):
        out[:, BLOC * c:BLOC * (c + 1)] = res[c]["out"]
    return out.reshape(A * B, S, H)



# revision 17
# speedup vs baseline: 1.4906x; 1.4906x over previous
"""CommNet message-passing kernel for Trainium2 (8 NeuronCores).

Problem (reference semantics):
    A, B, S, H = 8, 64, 1024, 128
    msg   = transpose(rnn_h, (2,1,0,3)) * alive            # (A,B,S,H)
    denom = max(sum_a alive, 1)                            # (1,B,S,1)
    msg   = msg / denom
    msg   = einsum('absh,oh->abso', msg, W) + b            # per-token HxH linear
    out   = obs + msg.reshape(A*B, S, H)

Sharding: data-parallel over the env-batch axis B (8 batches per core).
All ops are batch-local; W/b are replicated.

Per-core layout strategy (v2 — DMA-efficiency-first):
  The kernel is memory-bound (~100 MB/core HBM traffic).  The v1 layout
  put 128 tokens of one (a,b) pair on partitions with H on columns, which
  makes EVERY stream move in 512-byte chunks (~230 GB/s effective).  v2
  instead processes s-blocks of 16 sequence positions covering ALL 64
  (b,a) pairs at once (1024 tokens = 512 KB per stream per block):

  - rnn  (S,B,A,H): rows r = 64*s' + 8*b + a of a block are CONTIGUOUS in
    DRAM; tile [p=(s' b), (a h)] gives 4 KB contiguous per partition (the
    whole 512 KB block is one sequential read).
  - obs/out (A,B,S,H): tile [p=(a b sig), (t2 h)] with s = 16k + 8*sig + t2
    gives 4 KB contiguous per partition.
  - All three streams therefore DMA at full bus rate (~360 GB/s/core).

  Compute per block (msg path in bf16; tolerance is 2e-2, bf16 adds ~4e-4):
    - per agent a: DVE scales rnn sub-tile by alive/denom (per-partition
      scalars, pre-arranged layout), output bf16,
    - 8 PE transposes (1 cyc/row bf16) -> pa (h, token) in PSUM,
    - ScalarE copies pa -> SBUF mt,
    - one W-stationary bf16 matmul, N=1024 -> pb (o, token) f32 in PSUM,
    - ScalarE adds bias during pb -> SBUF ob copy (bf16),
    - 8 PE transposes back with strided column APs that simultaneously
      perform the (s,b,a) -> (a,b,s) token permute -> pc,
    - one DVE add pc + obs -> out tile, one 512 KB store.
  Scale = alive/max(sum_a alive,1) is computed on device once from a
  host-pre-permuted f32 copy of alive (DVE tree-sum + reciprocal).
"""

import os
import sys

import numpy as np

for _p in ("/opt/trn_rl_repo", "/root/.axon_site/_ro/trn_rl_repo"):
    if os.path.isdir(_p) and _p not in sys.path:
        sys.path.append(_p)

A, B, S, H = 8, 64, 1024, 128
NCORES = 8
BLOC = B // NCORES  # 8 env batches per core


def _build_program(s_len=S, transpose_dt="bfloat16", reps=1):
    """Build the per-core Bass program (identical on all cores).

    reps>1 repeats the whole main loop (same I/O) — used only for timing,
    since single-call wall time is dominated by ~70ms axon RTT."""
    import concourse.bass as bass  # noqa: F401
    import concourse.bacc as bacc
    import concourse.tile as tile
    from concourse import mybir

    f32 = mybir.dt.float32
    f32r = mybir.dt.float32r
    bf16 = mybir.dt.bfloat16

    assert s_len % 16 == 0
    nk = s_len // 16  # number of 16-seq blocks

    nc = bacc.Bacc("TRN2", target_bir_lowering=False, debug=False,
                   num_devices=NCORES)

    rnn = nc.dram_tensor("rnn", [s_len, BLOC, A, H], f32,
                         kind="ExternalInput").ap()
    obs = nc.dram_tensor("obs", [A, BLOC, s_len, H], f32,
                         kind="ExternalInput").ap()
    # pre-permuted f32 aliveness: alive_arr[8*s16 + b, k, a]
    #   = alive[a, b, 16*k + s16]
    alive = nc.dram_tensor("alive", [128, nk, 8], f32,
                           kind="ExternalInput").ap()
    wt = nc.dram_tensor("wt", [H, H], f32, kind="ExternalInput").ap()
    bias = nc.dram_tensor("bias", [H, 1], f32, kind="ExternalInput").ap()
    ident = nc.dram_tensor("ident", [128, 128], f32, kind="ExternalInput").ap()
    out = nc.dram_tensor("out", [A, BLOC, s_len, H], f32,
                         kind="ExternalOutput").ap()

    # block views; within block k:
    #   rnn partition p = 8*s' + b, columns (a, h)      -- 4KB runs
    #   obs/out partition p' = 16*a + 2*b + sig, columns (t2, h), s = 8*sig+t2
    rnn_r = rnn.rearrange("(k s) b a h -> k s b a h", s=16)
    obs_r = obs.rearrange("a b (k sig t) h -> k a b sig t h", sig=2, t=8)
    out_r = out.rearrange("a b (k sig t) h -> k a b sig t h", sig=2, t=8)

    tdt = {"float32": f32, "float32r": f32r,
           "bfloat16": bf16}[transpose_dt]
    mm_dt = bf16 if transpose_dt == "bfloat16" else f32r
    # PSUM banks per [128, 8, 128] tile: bf16 -> 1 bank, f32/f32r -> 2.
    pbufs = 2 if tdt == bf16 else 1

    with tile.TileContext(nc) as tc:
        with tc.tile_pool(name="consts", bufs=1) as consts, \
             tc.tile_pool(name="pre", bufs=1) as pre, \
             tc.tile_pool(name="rnnp", bufs=3) as rnn_pool, \
             tc.tile_pool(name="obsp", bufs=3) as obs_pool, \
             tc.tile_pool(name="outp", bufs=3) as out_pool, \
             tc.tile_pool(name="scaledp", bufs=2) as scaled_pool, \
             tc.tile_pool(name="mtp", bufs=2) as mt_pool, \
             tc.tile_pool(name="obp", bufs=2) as ob_pool, \
             tc.tile_pool(name="pap", bufs=pbufs, space="PSUM") as pa_pool, \
             tc.tile_pool(name="pbp", bufs=2, space="PSUM") as pb_pool, \
             tc.tile_pool(name="pcp", bufs=pbufs, space="PSUM") as pc_pool:

            # ---- constants ----
            wt_sb = consts.tile([128, 128], f32, tag="wt")
            nc.sync.dma_start(out=wt_sb, in_=wt)
            # matmul operands must be *produced* in their dtype (walrus
            # verifier); round W once on DVE.
            wt_r = consts.tile([128, 128], mm_dt, tag="wtr")
            nc.vector.tensor_copy(out=wt_r, in_=wt_sb)
            id_sb = consts.tile([128, 128], f32, tag="id")
            nc.sync.dma_start(out=id_sb, in_=ident)
            b_sb = consts.tile([128, 1], f32, tag="b")
            nc.sync.dma_start(out=b_sb, in_=bias)
            if tdt == f32:
                id_t = id_sb
            else:
                id_t = consts.tile([128, 128], tdt, tag="idt")
                nc.vector.tensor_copy(out=id_t, in_=id_sb)

            # ---- scale = alive / max(sum_a alive, 1), DVE only ----
            alive_sb = pre.tile([128, nk, 8], f32, tag="alive")
            nc.sync.dma_start(out=alive_sb, in_=alive)
            s4 = pre.tile([128, nk, 4], f32, tag="s4")
            nc.vector.tensor_add(out=s4, in0=alive_sb[:, :, 0:4],
                                 in1=alive_sb[:, :, 4:8])
            s2 = pre.tile([128, nk, 2], f32, tag="s2")
            nc.vector.tensor_add(out=s2, in0=s4[:, :, 0:2], in1=s4[:, :, 2:4])
            s1 = pre.tile([128, nk, 1], f32, tag="s1")
            nc.vector.tensor_add(out=s1, in0=s2[:, :, 0:1], in1=s2[:, :, 1:2])
            dmax = pre.tile([128, nk, 1], f32, tag="dmax")
            nc.vector.tensor_scalar_max(out=dmax, in0=s1, scalar1=1.0)
            rec = pre.tile([128, nk, 1], f32, tag="rec")
            nc.vector.reciprocal(out=rec, in_=dmax)
            scale_sb = pre.tile([128, nk, 8], f32, tag="scale")
            for a in range(A):
                nc.vector.tensor_mul(out=scale_sb[:, :, a:a + 1],
                                     in0=alive_sb[:, :, a:a + 1], in1=rec)

            # ---- main loop over 16-seq blocks ----
            ident_f = mybir.ActivationFunctionType.Identity
            for _rep in range(reps):
              for k in range(nk):
                # Spread DMA issue across sequencers: SP blocks once its
                # 4-deep wait queue fills with store DMAs, so loads issue
                # from SP/GpSimd and stores from ScalarE (timeline-sim
                # sweep: 414us -> 288us).
                rnn_t = rnn_pool.tile([128, 8, 128], f32, tag="rnn_t")
                nc.sync.dma_start(out=rnn_t, in_=rnn_r[k])
                obs_t = obs_pool.tile([128, 8, 128], f32, tag="obs_t")
                nc.gpsimd.dma_start(out=obs_t, in_=obs_r[k])

                scaled = scaled_pool.tile([128, 8, 128], tdt, tag="scaled")
                for a in range(A):
                    nc.vector.tensor_scalar_mul(
                        out=scaled[:, a, :], in0=rnn_t[:, a, :],
                        scalar1=scale_sb[:, k, a:a + 1])

                # PSUM accumulation groups cannot span banks (2KB/partition):
                # group size 8 sub-tiles for 2-byte dtypes, 4 for 4-byte.
                grp = 8 if mybir.dt.size(tdt) == 2 else 4
                pa = pa_pool.tile([128, 8, 128], tdt, tag="pa")
                for a in range(A):
                    nc.tensor.matmul(out=pa[:, a, :], lhsT=scaled[:, a, :],
                                     rhs=id_t, is_transpose=True,
                                     start=(a % grp == 0),
                                     stop=(a % grp == grp - 1))
                mt = mt_pool.tile([128, 8, 128], mm_dt, tag="mt")
                nc.scalar.copy(out=mt, in_=pa)

                mt_f = mt.rearrange("p a h -> p (a h)")
                pb = pb_pool.tile([128, 1024], f32, tag="pb")
                for hh in range(2):
                    nc.tensor.matmul(out=pb[:, 512 * hh:512 * (hh + 1)],
                                     lhsT=wt_r,
                                     rhs=mt_f[:, 512 * hh:512 * (hh + 1)],
                                     start=True, stop=True)
                # ob[o, t2, a, b, sig]: flat col = 128*t2 + 16a + 2b + sig.
                # The bias-copy permutes from pb's token order (a, sig, t2,
                # b) so each ob[:, t2] is a CONTIGUOUS 128-col transpose
                # operand whose column order (a, b, sig) equals the store
                # partition order of out_r/obs_r.
                # (ACT ISA caps free dims at 3 -> split the permuted
                # bias-copy over sig.)
                ob = ob_pool.tile([128, 8, 8, 8, 2], tdt, tag="ob")
                ob_p = ob.rearrange("o t a b g -> o g a t b")
                pb_p = pb.rearrange("o (a g t b) -> o g a t b",
                                    a=8, g=2, t=8)
                for g in range(2):
                    nc.scalar.activation(
                        out=ob_p[:, g], in_=pb_p[:, g],
                        func=ident_f, bias=b_sb, scale=1.0)

                ob_v = ob.rearrange("o t a b g -> o t (a b g)")
                pc = pc_pool.tile([128, 8, 128], tdt, tag="pc")
                for t2 in range(8):
                    nc.tensor.matmul(out=pc[:, t2, :], lhsT=ob_v[:, t2],
                                     rhs=id_t, is_transpose=True,
                                     start=(t2 % grp == 0),
                                     stop=(t2 % grp == grp - 1))

                out_t = out_pool.tile([128, 8, 128], f32, tag="out_t")
                nc.vector.tensor_add(
                    out=out_t.rearrange("p t h -> p (t h)"),
                    in0=pc.rearrange("p t h -> p (t h)"),
                    in1=obs_t.rearrange("p t h -> p (t h)"))
                nc.scalar.dma_start(out=out_r[k], in_=out_t)
    nc.compile()
    return nc


def make_in_maps(obs, rnn_h, alive, W, b, s_len=S):
    """Shard full inputs into per-core input maps (host-side slicing only)."""
    obs4 = obs.reshape(A, B, S, H)
    nk = s_len // 16
    wt = np.ascontiguousarray(W.T.astype(np.float32))
    b2 = np.ascontiguousarray(b.astype(np.float32).reshape(H, 1))
    ident = np.eye(128, dtype=np.float32)
    in_maps = []
    for c in range(NCORES):
        bs = slice(BLOC * c, BLOC * (c + 1))
        al = alive[:, bs, :s_len, 0]  # (A, 8, s_len) int32
        # alive_arr[8*s16 + b, k, a] = alive[a, b, 16k + s16]
        al_arr = np.ascontiguousarray(
            al.reshape(A, BLOC, nk, 16).transpose(3, 1, 2, 0)
            .reshape(128, nk, A).astype(np.float32))
        in_maps.append({
            "rnn": np.ascontiguousarray(rnn_h[:s_len, bs]),
            "obs": np.ascontiguousarray(obs4[:, bs, :s_len]),
            "alive": al_arr,
            "wt": wt, "bias": b2, "ident": ident,
        })
    return in_maps


_NC_CACHE = {}


def get_nc(s_len=S, transpose_dt=None, reps=1):
    if transpose_dt is None:
        transpose_dt = DEFAULT_TRANSPOSE_DT
    key = (s_len, transpose_dt, reps)
    if key not in _NC_CACHE:
        _NC_CACHE[key] = _build_program(s_len, transpose_dt, reps)
    return _NC_CACHE[key]


DEFAULT_TRANSPOSE_DT = "bfloat16"


def kernel(obs, rnn_h, alive, W, b):
    from concourse.bass_utils import run_bass_kernel_spmd

    nc = get_nc(S, DEFAULT_TRANSPOSE_DT)
    in_maps = make_in_maps(obs, rnn_h, alive, W, b)
    res = run_bass_kernel_spmd(nc, in_maps, list(range(NCORES))).results
    out = np.empty((A, B, S, H), np.float32)
    for c in range(NCORES):
        out[:, BLOC * c:BLOC * (c + 1)] = res[c]["out"]
    return out.reshape(A * B, S, H)
